# revision 20
# baseline (speedup 1.0000x reference)
"""MoE pointwise conv2d kernel for Trainium2 (8 NeuronCores, SPMD data-parallel).

Problem: out[b,o,h,w] = sum_i (sum_e routing[b,e] * weight[e,o,i]) * x[b,i,h,w]
Shapes:  x [64,384,28,28] f32, routing [64,8] f32, weight [8,384,384] f32.

Strategy (per core, 8 samples each), fp16 wire format end-to-end:
  - Routing-combine runs on TensorE (measured DVE scalar_tensor_tensor runs
    1x-mode only => a DVE MAC chain costs ~71us/core; TensorE does the same
    contraction in ~8us):
      The host expands routing into a sparse matrix
        rq[(e,o16), (b,o16')] = r[b,e] * delta(o16,o16')   [128 x 128]
      and pre-permutes weights to
        wt[(e,o16), (ki, chunk, i_lo)]                     [128 x 9216]
      so one matmul per (ki, o-chunk of 16) computes
        agg^T[i_lo, (b, o16)] = sum_e r[b,e] w[e, chunk*16+o16, ki*128+i_lo]
      for ALL 8 samples at once: 72 matmuls, FD=128, fp32 PSUM accumulate.
  - ScalarE evacuates agg psum tiles ([128,512], 4 chunks each) into a
    [128, 9216] f16 staging tile laid out (ki, chunk, b, o16).
  - Main GEMM out[b] = agg_b @ x_b on TensorE: lhsT tiles are strided 3D APs
    into staging (8 chunks x 16 cols per (ki,mo,b)); psum [128,784] spanning
    2 banks, accumulation groups FD 512 + 272 over 3 k-tiles.
  - PSUM out evacuation alternates ScalarE/VectorE; one [128, 3*784] out
    tile per sample.
  - DMAs per rep: 6 wt (split by ki x 2) + 1 rq + 8 x + 10 out = 25
    (last sample's out DMA is split per-mo to trim the tail); merged
    multi-dim access patterns keep the ~0.6us/DMA HWDGE+SP dispatch cost
    off the critical path (the 2-DMA-per-tile layout had 81).
"""
import os
import sys

sys.path.insert(0, "/opt/trn_rl_repo")

import numpy as np
from contextlib import ExitStack

B, C_IN, C_OUT, E, H, W = 64, 384, 384, 8, 28, 28
HW = H * W            # 784
N_CORES = 8
BPC = B // N_CORES    # 8 samples per core
KI = C_IN // 128      # 3 k-tiles
MO = C_OUT // 128     # 3 output-partition tiles
OC = 16               # o-values per chunk
NCH = C_OUT // OC     # 24 o-chunks
CPK = NCH * 128       # staging cols per ki (3072)
SCOL = KI * CPK       # staging cols total (9216)
NSPLITS = ((0, 512), (512, 272))  # psum accumulation groups (bank-aligned)

_cache = {}


def _build(reps=1, serialize_reps=False, small_out=False, cg4=4,
           evac_split=True, wt_splits=2, out_split_mo=False,
           agg_evac_split=False):
    import concourse.tile as tile
    import concourse.mybir as mybir
    from concourse import bacc
    from concourse.tile import add_dep_helper

    f32 = mybir.dt.float32
    f16 = mybir.dt.float16

    nc = bacc.Bacc("TRN2", target_bir_lowering=False, debug=False)
    x_d = nc.dram_tensor("x", [BPC, KI, 128, HW], f16, kind="ExternalInput")
    rq_d = nc.dram_tensor("rq", [128, 128], f16, kind="ExternalInput")
    wt_d = nc.dram_tensor("wt", [KI, 128, CPK], f16, kind="ExternalInput")
    out_d = nc.dram_tensor("out", [(1 if small_out else reps) * BPC, MO, 128, HW],
                           f16, kind="ExternalOutput")

    with tile.TileContext(nc) as tc:
        with ExitStack() as ctx:
            wt_pool = ctx.enter_context(tc.tile_pool(name="wt", bufs=2))
            rq_pool = ctx.enter_context(tc.tile_pool(name="rq", bufs=2))
            stag_pool = ctx.enter_context(tc.tile_pool(name="st", bufs=2))
            x_pool = ctx.enter_context(tc.tile_pool(name="xp", bufs=3))
            out_pool = ctx.enter_context(tc.tile_pool(name="op", bufs=3))
            psa_pool = ctx.enter_context(tc.tile_pool(name="pa", bufs=2,
                                                      space="PSUM"))
            psm_pool = ctx.enter_context(tc.tile_pool(name="pm", bufs=3,
                                                      space="PSUM"))

            prev_out_dmas, cur_out_dmas = [], []

            def _fence(inst):
                if serialize_reps:
                    for d in prev_out_dmas:
                        add_dep_helper(inst.ins, d.ins, reason="serialize reps")
                return inst

            for rep in range(reps):
                prev_out_dmas, cur_out_dmas = cur_out_dmas, []
                rq_sb = rq_pool.tile([128, 128], f16)
                _fence(nc.sync.dma_start(rq_sb[:], rq_d[:]))
                wt_sb = wt_pool.tile([128, SCOL], f16)
                wt_dmas = []
                csz = CPK // wt_splits
                for ki in range(KI):
                    for s in range(wt_splits):
                        wt_dmas.append(_fence(nc.sync.dma_start(
                            wt_sb[:, ki * CPK + s * csz:
                                  ki * CPK + (s + 1) * csz],
                            wt_d[ki, :, s * csz:(s + 1) * csz])))

                # ---- routing-combine on TensorE ----
                # stag[(ki, chunk, o16, b)] = agg[b, chunk*16+o16, ki*128+p]
                stag = stag_pool.tile([128, SCOL], f16)
                for ki in range(KI):
                    for cg in range(NCH // cg4):
                        ps = psa_pool.tile([128, cg4 * 128], f32)
                        for c4 in range(cg4):
                            chunk = cg * cg4 + c4
                            nc.tensor.matmul(
                                ps[:, c4 * 128:(c4 + 1) * 128],
                                wt_sb[:, (ki * NCH + chunk) * 128:
                                      (ki * NCH + chunk + 1) * 128],
                                rq_sb[:],
                                start=True, stop=True,
                            )
                        base = (ki * NCH + cg * cg4) * 128
                        if agg_evac_split and cg % 2 == 1:
                            nc.vector.tensor_copy(
                                stag[:, base:base + cg4 * 128], ps[:])
                        else:
                            nc.scalar.copy(stag[:, base:base + cg4 * 128],
                                           ps[:])

                # ---- per-sample GEMM + evac + out DMA ----
                for b in range(BPC):
                    x_sb = x_pool.tile([128, KI, HW], f16)
                    xi = _fence(nc.sync.dma_start(
                        x_sb[:], x_d[b].transpose([1, 0, 2])))
                    if b < 2:
                        for wd in wt_dmas:
                            add_dep_helper(xi.ins, wd.ins,
                                           reason="x after wt (head trim)")
                    o_sb = out_pool.tile([128, MO, HW], f16)
                    for mo in range(MO):
                        ps = psm_pool.tile([128, HW], f32)
                        for n0, nw in NSPLITS:
                            for ki in range(KI):
                                base = (ki * NCH + mo * (NCH // MO)) * 128
                                lhs = stag[:, base + b:base + 1024:BPC]
                                nc.tensor.matmul(
                                    ps[:, n0:n0 + nw],
                                    lhs, x_sb[:, ki, n0:n0 + nw],
                                    start=(ki == 0), stop=(ki == KI - 1),
                                )
                        if evac_split and mo >= 1:
                            nc.vector.tensor_copy(o_sb[:, mo, :], ps[:])
                        else:
                            nc.scalar.copy(o_sb[:, mo, :], ps[:])
                        if out_split_mo or b == BPC - 1:
                            cur_out_dmas.append(nc.sync.dma_start(
                                out_d[(0 if small_out else rep) * BPC + b,
                                      mo], o_sb[:, mo, :]))
                    if not (out_split_mo or b == BPC - 1):
                        cur_out_dmas.append(nc.sync.dma_start(
                            out_d[(0 if small_out else rep) * BPC + b]
                            .transpose([1, 0, 2]), o_sb[:]))
    nc.compile()
    return nc


def _host_prep(x, routing_weights, weight):
    """Full inputs -> per-core in_maps with the kernel's dram layouts."""
    # wt[ki][e*16+o16, chunk*128 + i_lo] = weight[e, chunk*16+o16, ki*128+i_lo]
    wt = np.ascontiguousarray(
        weight.reshape(E, NCH, OC, KI, 128)      # e, chunk, o16, ki, i_lo
        .transpose(3, 0, 2, 1, 4)                # ki, e, o16, chunk, i_lo
        .reshape(KI, 128, CPK).astype(np.float16))
    x_r = np.ascontiguousarray(x.reshape(B, KI, 128, HW).astype(np.float16))

    in_maps = []
    for c in range(N_CORES):
        r_core = routing_weights[c * BPC:(c + 1) * BPC]   # [BPC, E]
        rq = np.zeros((E, OC, OC, BPC), dtype=np.float16)
        for o16 in range(OC):
            rq[:, o16, o16, :] = r_core.T.astype(np.float16)
        in_maps.append({
            "x": x_r[c * BPC:(c + 1) * BPC],
            "rq": np.ascontiguousarray(rq.reshape(128, 128)),
            "wt": wt,
        })
    return in_maps


def kernel(x: np.ndarray, routing_weights: np.ndarray, weight: np.ndarray,
           _trace: bool = False):
    from concourse.bass_utils import run_bass_kernel_spmd

    x = np.asarray(x, dtype=np.float32)
    routing_weights = np.ascontiguousarray(
        np.asarray(routing_weights, dtype=np.float32))
    weight = np.asarray(weight, dtype=np.float32)

    if "nc" not in _cache:
        _cache["nc"] = _build()
    nc = _cache["nc"]

    in_maps = _host_prep(x, routing_weights, weight)
    res = run_bass_kernel_spmd(nc, in_maps, core_ids=list(range(N_CORES)),
                               trace=_trace)
    out = np.concatenate([res.results[c]["out"] for c in range(N_CORES)],
                         axis=0)
    if _trace:
        _cache["last_result"] = res
    return out.reshape(B, C_OUT, H, W).astype(np.float32)


if __name__ == "__main__":
    rng = np.random.default_rng(0)
    x = rng.standard_normal((B, C_IN, H, W), dtype=np.float32)
    rw = rng.random((B, E), dtype=np.float32)
    w = rng.standard_normal((E, C_OUT, C_IN), dtype=np.float32)
    got = kernel(x, rw, w)
    agg = np.einsum('be,eoi->boi', rw, w)
    want = np.einsum('boi,bihw->bohw', agg, x.reshape(B, C_IN, H, W))
    err = np.abs(got - want).max() / np.abs(want).max()
    print("rel err:", err)


# revision 28
# speedup vs baseline: 1.0453x; 1.0453x over previous
"""MoE pointwise conv2d kernel for Trainium2 (8 NeuronCores, SPMD data-parallel).

Problem: out[b,o,h,w] = sum_i (sum_e routing[b,e] * weight[e,o,i]) * x[b,i,h,w]
Shapes:  x [64,384,28,28] f32, routing [64,8] f32, weight [8,384,384] f32.

Strategy (per core, 8 samples each), fp16 wire format end-to-end:
  - Routing-combine runs on TensorE (measured DVE scalar_tensor_tensor runs
    1x-mode only => a DVE MAC chain costs ~71us/core; TensorE does the same
    contraction in ~8us):
      The host expands routing into a sparse matrix
        rq[(e,o16), (b,o16')] = r[b,e] * delta(o16,o16')   [128 x 128]
      and pre-permutes weights to
        wt[(e,o16), (ki, chunk, i_lo)]                     [128 x 9216]
      so one matmul per (ki, o-chunk of 16) computes
        agg^T[i_lo, (b, o16)] = sum_e r[b,e] w[e, chunk*16+o16, ki*128+i_lo]
      for ALL 8 samples at once: 72 matmuls, FD=128, fp32 PSUM accumulate.
  - ScalarE evacuates agg psum tiles ([128,512], 4 chunks each) into a
    [128, 9216] f16 staging tile laid out (ki, chunk, b, o16).
  - Main GEMM out[b] = agg_b @ x_b on TensorE: lhsT tiles are strided 3D APs
    into staging (8 chunks x 16 cols per (ki,mo,b)); psum [128,784] spanning
    2 banks, accumulation groups FD 512 + 272 over 3 k-tiles.
  - PSUM out evacuation alternates ScalarE/VectorE; one [128, 3*784] out
    tile per sample.
  - DMAs per rep: 6 wt (split by ki x 2) + 1 rq + 8 x + 10 out = 25
    (last sample's out DMA is split per-mo to trim the tail); merged
    multi-dim access patterns keep the ~0.6us/DMA HWDGE+SP dispatch cost
    off the critical path (the 2-DMA-per-tile layout had 81).
"""
import os
import sys

sys.path.insert(0, "/opt/trn_rl_repo")

import numpy as np
from contextlib import ExitStack

B, C_IN, C_OUT, E, H, W = 64, 384, 384, 8, 28, 28
HW = H * W            # 784
N_CORES = 8
BPC = B // N_CORES    # 8 samples per core
KI = C_IN // 128      # 3 k-tiles
MO = C_OUT // 128     # 3 output-partition tiles
OC = 16               # o-values per chunk
NCH = C_OUT // OC     # 24 o-chunks
CPK = NCH * 128       # staging cols per ki (3072)
SCOL = KI * CPK       # staging cols total (9216)
NSPLITS = ((0, 512), (512, 272))  # psum accumulation groups (bank-aligned)

_cache = {}


def _build(reps=1, serialize_reps=False, small_out=False, cg4=4,
           evac_split=True, wt_splits=2, out_split_mo=False,
           agg_evac_split=False, psm_split=False, deep_bufs=4,
           agg_evac_pair=False, wt_head=True):
    import concourse.tile as tile
    import concourse.mybir as mybir
    from concourse import bacc
    from concourse.tile import add_dep_helper

    f32 = mybir.dt.float32
    f16 = mybir.dt.float16

    nc = bacc.Bacc("TRN2", target_bir_lowering=False, debug=False)
    x_d = nc.dram_tensor("x", [BPC, KI, 128, HW], f16, kind="ExternalInput")
    rq_d = nc.dram_tensor("rq", [128, 128], f16, kind="ExternalInput")
    wt_d = nc.dram_tensor("wt", [KI, 128, CPK], f16, kind="ExternalInput")
    out_d = nc.dram_tensor("out", [(1 if small_out else reps) * BPC, MO, 128, HW],
                           f16, kind="ExternalOutput")

    with tile.TileContext(nc) as tc:
        with ExitStack() as ctx:
            wt_pool = ctx.enter_context(tc.tile_pool(name="wt", bufs=2))
            rq_pool = ctx.enter_context(tc.tile_pool(name="rq", bufs=2))
            stag_pool = ctx.enter_context(tc.tile_pool(name="st", bufs=2))
            nbuf = deep_bufs if isinstance(deep_bufs, int) and deep_bufs > 1 \
                else (4 if deep_bufs else 3)
            x_pool = ctx.enter_context(tc.tile_pool(name="xp", bufs=nbuf))
            out_pool = ctx.enter_context(tc.tile_pool(name="op", bufs=nbuf))
            psa_pool = ctx.enter_context(tc.tile_pool(name="pa", bufs=2,
                                                      space="PSUM"))
            psm_pool = ctx.enter_context(tc.tile_pool(
                name="pm", bufs=3 if psm_split else 3, space="PSUM"))
            psm2_pool = ctx.enter_context(tc.tile_pool(
                name="pm2", bufs=3, space="PSUM")) if psm_split else None

            prev_out_dmas, cur_out_dmas = [], []

            def _fence(inst):
                if serialize_reps:
                    for d in prev_out_dmas:
                        add_dep_helper(inst.ins, d.ins, reason="serialize reps")
                return inst

            for rep in range(reps):
                prev_out_dmas, cur_out_dmas = cur_out_dmas, []
                rq_sb = rq_pool.tile([128, 128], f16)
                _fence(nc.sync.dma_start(rq_sb[:], rq_d[:]))
                wt_sb = wt_pool.tile([128, SCOL], f16)
                wt_dmas = []
                csz = CPK // wt_splits
                pieces = []
                for ki in range(KI):
                    lo = 0
                    if wt_head and ki == 0:
                        pieces.append((0, 0, 512))
                        lo = 512
                    for s in range(wt_splits):
                        hi = (s + 1) * csz
                        if hi > lo:
                            pieces.append((ki, lo, hi))
                            lo = hi
                for ki, lo, hi in pieces:
                    wt_dmas.append(_fence(nc.sync.dma_start(
                        wt_sb[:, ki * CPK + lo:ki * CPK + hi],
                        wt_d[ki, :, lo:hi])))

                # ---- routing-combine on TensorE ----
                # stag[(ki, chunk, o16, b)] = agg[b, chunk*16+o16, ki*128+p]
                stag = stag_pool.tile([128, SCOL], f16)
                for ki in range(KI):
                    for cg in range(NCH // cg4):
                        ps = psa_pool.tile([128, cg4 * 128], f32)
                        for c4 in range(cg4):
                            chunk = cg * cg4 + c4
                            nc.tensor.matmul(
                                ps[:, c4 * 128:(c4 + 1) * 128],
                                wt_sb[:, (ki * NCH + chunk) * 128:
                                      (ki * NCH + chunk + 1) * 128],
                                rq_sb[:],
                                start=True, stop=True,
                            )
                        base = (ki * NCH + cg * cg4) * 128
                        half = cg4 * 128 // 2
                        if agg_evac_pair:
                            nc.scalar.copy(stag[:, base:base + half],
                                           ps[:, 0:half])
                            nc.vector.tensor_copy(
                                stag[:, base + half:base + cg4 * 128],
                                ps[:, half:cg4 * 128])
                        elif agg_evac_split and cg % 2 == 1:
                            nc.vector.tensor_copy(
                                stag[:, base:base + cg4 * 128], ps[:])
                        else:
                            nc.scalar.copy(stag[:, base:base + cg4 * 128],
                                           ps[:])

                # ---- per-sample GEMM + evac + out DMA ----
                for b in range(BPC):
                    x_sb = x_pool.tile([128, KI, HW], f16)
                    xi = _fence(nc.sync.dma_start(
                        x_sb[:], x_d[b].transpose([1, 0, 2])))
                    if b < 2:
                        for wd in wt_dmas:
                            add_dep_helper(xi.ins, wd.ins,
                                           reason="x after wt (head trim)")
                    o_sb = out_pool.tile([128, MO, HW], f16)
                    for mo in range(MO):
                        if psm_split:
                            ps_a = psm_pool.tile([128, 512], f32)
                            ps_b = psm2_pool.tile([128, HW - 512], f32)
                            segs = ((0, 512, ps_a), (512, HW - 512, ps_b))
                        else:
                            ps = psm_pool.tile([128, HW], f32)
                            segs = tuple((n0, nw, ps[:, n0:n0 + nw])
                                         for n0, nw in NSPLITS)
                        for n0, nw, pseg in segs:
                            for ki in range(KI):
                                base = (ki * NCH + mo * (NCH // MO)) * 128
                                lhs = stag[:, base + b:base + 1024:BPC]
                                nc.tensor.matmul(
                                    pseg[:] if psm_split else pseg,
                                    lhs, x_sb[:, ki, n0:n0 + nw],
                                    start=(ki == 0), stop=(ki == KI - 1),
                                )
                        if psm_split:
                            eng = (nc.vector.tensor_copy
                                   if evac_split and mo >= 1 else nc.scalar.copy)
                            eng(o_sb[:, mo, 0:512], ps_a[:])
                            eng(o_sb[:, mo, 512:HW], ps_b[:])
                        elif evac_split and mo >= 1:
                            nc.vector.tensor_copy(o_sb[:, mo, :], ps[:])
                        else:
                            nc.scalar.copy(o_sb[:, mo, :], ps[:])
                        if out_split_mo or b == BPC - 1:
                            cur_out_dmas.append(nc.sync.dma_start(
                                out_d[(0 if small_out else rep) * BPC + b,
                                      mo], o_sb[:, mo, :]))
                    if not (out_split_mo or b == BPC - 1):
                        cur_out_dmas.append(nc.sync.dma_start(
                            out_d[(0 if small_out else rep) * BPC + b]
                            .transpose([1, 0, 2]), o_sb[:]))
    nc.compile()
    return nc


def _host_prep(x, routing_weights, weight):
    """Full inputs -> per-core in_maps with the kernel's dram layouts."""
    # wt[ki][e*16+o16, chunk*128 + i_lo] = weight[e, chunk*16+o16, ki*128+i_lo]
    wt = np.ascontiguousarray(
        weight.reshape(E, NCH, OC, KI, 128)      # e, chunk, o16, ki, i_lo
        .transpose(3, 0, 2, 1, 4)                # ki, e, o16, chunk, i_lo
        .reshape(KI, 128, CPK).astype(np.float16))
    x_r = np.ascontiguousarray(x.reshape(B, KI, 128, HW).astype(np.float16))

    in_maps = []
    for c in range(N_CORES):
        r_core = routing_weights[c * BPC:(c + 1) * BPC]   # [BPC, E]
        rq = np.zeros((E, OC, OC, BPC), dtype=np.float16)
        for o16 in range(OC):
            rq[:, o16, o16, :] = r_core.T.astype(np.float16)
        in_maps.append({
            "x": x_r[c * BPC:(c + 1) * BPC],
            "rq": np.ascontiguousarray(rq.reshape(128, 128)),
            "wt": wt,
        })
    return in_maps


def kernel(x: np.ndarray, routing_weights: np.ndarray, weight: np.ndarray,
           _trace: bool = False):
    from concourse.bass_utils import run_bass_kernel_spmd

    x = np.asarray(x, dtype=np.float32)
    routing_weights = np.ascontiguousarray(
        np.asarray(routing_weights, dtype=np.float32))
    weight = np.asarray(weight, dtype=np.float32)

    if "nc" not in _cache:
        _cache["nc"] = _build()
    nc = _cache["nc"]

    in_maps = _host_prep(x, routing_weights, weight)
    res = run_bass_kernel_spmd(nc, in_maps, core_ids=list(range(N_CORES)),
                               trace=_trace)
    out = np.concatenate([res.results[c]["out"] for c in range(N_CORES)],
                         axis=0)
    if _trace:
        _cache["last_result"] = res
    return out.reshape(B, C_OUT, H, W).astype(np.float32)


if __name__ == "__main__":
    rng = np.random.default_rng(0)
    x = rng.standard_normal((B, C_IN, H, W), dtype=np.float32)
    rw = rng.random((B, E), dtype=np.float32)
    w = rng.standard_normal((E, C_OUT, C_IN), dtype=np.float32)
    got = kernel(x, rw, w)
    agg = np.einsum('be,eoi->boi', rw, w)
    want = np.einsum('boi,bihw->bohw', agg, x.reshape(B, C_IN, H, W))
    err = np.abs(got - want).max() / np.abs(want).max()
    print("rel err:", err)


# revision 36
# speedup vs baseline: 1.2607x; 1.2061x over previous
"""MoE pointwise conv2d kernel for Trainium2 (8 NeuronCores, SPMD data-parallel).

Problem: out[b,o,h,w] = sum_i (sum_e routing[b,e] * weight[e,o,i]) * x[b,i,h,w]
Shapes:  x [64,384,28,28] f32, routing [64,8] f32, weight [8,384,384] f32.

Strategy (per core, 8 samples each), fp16 wire format end-to-end:
  - Routing-combine runs on TensorE (measured DVE scalar_tensor_tensor runs
    1x-mode only => a DVE MAC chain costs ~71us/core; TensorE does the same
    contraction in ~8us):
      The host expands routing into a sparse matrix
        rq[(e,o16), (b,o16')] = r[b,e] * delta(o16,o16')   [128 x 128]
      and pre-permutes weights to
        wt[(e,o16), (ki, chunk, i_lo)]                     [128 x 9216]
      so one matmul per (ki, o-chunk of 16) computes
        agg^T[i_lo, (b, o16)] = sum_e r[b,e] w[e, chunk*16+o16, ki*128+i_lo]
      for ALL 8 samples at once: 72 matmuls, FD=128, fp32 PSUM accumulate.
  - ScalarE evacuates agg psum tiles ([128,512], 4 chunks each) into a
    [128, 9216] f16 staging tile laid out (ki, chunk, b, o16).
  - Main GEMM out[b] = agg_b @ x_b on TensorE: lhsT tiles are strided 3D APs
    into staging (8 chunks x 16 cols per (ki,mo,b)); psum [128,784] spanning
    2 banks, accumulation groups FD 512 + 272 over 3 k-tiles.
  - PSUM out evacuation alternates ScalarE/VectorE; one [128, 3*784] out
    tile per sample.
  - DMAs per rep: 6 wt (split by ki x 2) + 1 rq + 8 x + 10 out = 25
    (last sample's out DMA is split per-mo to trim the tail); merged
    multi-dim access patterns keep the ~0.6us/DMA HWDGE+SP dispatch cost
    off the critical path (the 2-DMA-per-tile layout had 81).
"""
import os
import sys

sys.path.insert(0, "/opt/trn_rl_repo")

import numpy as np
from contextlib import ExitStack

B, C_IN, C_OUT, E, H, W = 64, 384, 384, 8, 28, 28
HW = H * W            # 784
N_CORES = 8
BPC = B // N_CORES    # 8 samples per core
KI = C_IN // 128      # 3 k-tiles
MO = C_OUT // 128     # 3 output-partition tiles
OC = 16               # o-values per chunk
NCH = C_OUT // OC     # 24 o-chunks
CPK = NCH * 128       # staging cols per ki (3072)
SCOL = KI * CPK       # staging cols total (9216)
NSPLITS = ((0, 512), (512, 272))  # psum accumulation groups (bank-aligned)

_cache = {}


def _build(reps=1, serialize_reps=False, small_out=False, cg4=4,
           evac_split=True, wt_splits=2, out_split_mo=False,
           agg_evac_split=False, psm_split=False, deep_bufs=4,
           agg_evac_pair=False, wt_head=True, agg_order="ki",
           main_hybrid=0):
    import concourse.tile as tile
    import concourse.mybir as mybir
    from concourse import bacc
    from concourse.tile import add_dep_helper

    f32 = mybir.dt.float32
    f16 = mybir.dt.float16

    nc = bacc.Bacc("TRN2", target_bir_lowering=False, debug=False)
    x_d = nc.dram_tensor("x", [BPC, KI, 128, HW], f16, kind="ExternalInput")
    rq_d = nc.dram_tensor("rq", [128, 128], f16, kind="ExternalInput")
    wt_d = nc.dram_tensor("wt", [KI, 128, CPK], f16, kind="ExternalInput")
    out_d = nc.dram_tensor("out", [(1 if small_out else reps) * BPC, MO, 128, HW],
                           f16, kind="ExternalOutput")

    with tile.TileContext(nc) as tc:
        with ExitStack() as ctx:
            wt_pool = ctx.enter_context(tc.tile_pool(name="wt", bufs=2))
            rq_pool = ctx.enter_context(tc.tile_pool(name="rq", bufs=2))
            stag_pool = ctx.enter_context(tc.tile_pool(name="st", bufs=2))
            nbuf = deep_bufs if isinstance(deep_bufs, int) and deep_bufs > 1 \
                else (4 if deep_bufs else 3)
            x_pool = ctx.enter_context(tc.tile_pool(name="xp", bufs=nbuf))
            out_pool = ctx.enter_context(tc.tile_pool(name="op", bufs=nbuf))
            psa_pool = ctx.enter_context(tc.tile_pool(name="pa", bufs=2,
                                                      space="PSUM"))
            psm_pool = ctx.enter_context(tc.tile_pool(
                name="pm", bufs=3 if psm_split else 3, space="PSUM"))
            psm2_pool = ctx.enter_context(tc.tile_pool(
                name="pm2", bufs=3, space="PSUM")) if psm_split else None

            prev_out_dmas, cur_out_dmas = [], []

            def _fence(inst):
                if serialize_reps:
                    for d in prev_out_dmas:
                        add_dep_helper(inst.ins, d.ins, reason="serialize reps")
                return inst

            for rep in range(reps):
                prev_out_dmas, cur_out_dmas = cur_out_dmas, []
                rq_sb = rq_pool.tile([128, 128], f16)
                _fence(nc.sync.dma_start(rq_sb[:], rq_d[:]))
                wt_sb = wt_pool.tile([128, SCOL], f16)
                wt_dmas = []
                pieces = []
                if agg_order == "cg":
                    # 512-col pieces in (cg, ki) order: each piece lands just
                    # before the agg matmul group that consumes it
                    psz = cg4 * 128
                    for s in range(CPK // psz):
                        for ki in range(KI):
                            pieces.append((ki, s * psz, (s + 1) * psz))
                else:
                    csz = CPK // wt_splits
                    for ki in range(KI):
                        lo = 0
                        if wt_head and ki == 0:
                            pieces.append((0, 0, 512))
                            lo = 512
                        for s in range(wt_splits):
                            hi = (s + 1) * csz
                            if hi > lo:
                                pieces.append((ki, lo, hi))
                                lo = hi
                for ki, lo, hi in pieces:
                    wt_dmas.append(_fence(nc.sync.dma_start(
                        wt_sb[:, ki * CPK + lo:ki * CPK + hi],
                        wt_d[ki, :, lo:hi])))

                # ---- routing-combine on TensorE ----
                # stag[(ki, chunk, o16, b)] = agg[b, chunk*16+o16, ki*128+p]
                stag = stag_pool.tile([128, SCOL], f16)
                if agg_order == "cg":
                    order = [(ki, cg) for cg in range(NCH // cg4)
                             for ki in range(KI)]
                else:
                    order = [(ki, cg) for ki in range(KI)
                             for cg in range(NCH // cg4)]
                for ki, cg in order:
                    if True:
                        ps = psa_pool.tile([128, cg4 * 128], f32)
                        for c4 in range(cg4):
                            chunk = cg * cg4 + c4
                            nc.tensor.matmul(
                                ps[:, c4 * 128:(c4 + 1) * 128],
                                wt_sb[:, (ki * NCH + chunk) * 128:
                                      (ki * NCH + chunk + 1) * 128],
                                rq_sb[:],
                                start=True, stop=True,
                            )
                        base = (ki * NCH + cg * cg4) * 128
                        half = cg4 * 128 // 2
                        if agg_evac_pair:
                            nc.scalar.copy(stag[:, base:base + half],
                                           ps[:, 0:half])
                            nc.vector.tensor_copy(
                                stag[:, base + half:base + cg4 * 128],
                                ps[:, half:cg4 * 128])
                        elif agg_evac_split and cg % 2 == 1:
                            nc.vector.tensor_copy(
                                stag[:, base:base + cg4 * 128], ps[:])
                        else:
                            nc.scalar.copy(stag[:, base:base + cg4 * 128],
                                           ps[:])

                # ---- per-sample GEMM + evac + out DMA ----
                # First `main_hybrid` samples run mo-interleaved so early
                # units only need the agg evacs that have already drained.
                H = min(main_hybrid, BPC)
                units = ([(b, mo) for mo in range(MO) for b in range(H)]
                         + [(b, mo) for b in range(H, BPC)
                            for mo in range(MO)])
                x_sbs, o_sbs = {}, {}
                for b, mo in units:
                    if b not in x_sbs:
                        x_sb = x_pool.tile([128, KI, HW], f16, tag="x")
                        xi = _fence(nc.sync.dma_start(
                            x_sb[:], x_d[b].transpose([1, 0, 2])))
                        if b < 2:
                            for wd in wt_dmas:
                                add_dep_helper(xi.ins, wd.ins,
                                               reason="x after wt (head trim)")
                        x_sbs[b] = x_sb
                        o_sb = out_pool.tile([128, MO, HW], f16, tag="o")
                        o_sbs[b] = o_sb
                    x_sb, o_sb = x_sbs[b], o_sbs[b]
                    if True:
                        if psm_split:
                            ps_a = psm_pool.tile([128, 512], f32)
                            ps_b = psm2_pool.tile([128, HW - 512], f32)
                            segs = ((0, 512, ps_a), (512, HW - 512, ps_b))
                        else:
                            ps = psm_pool.tile([128, HW], f32)
                            segs = tuple((n0, nw, ps[:, n0:n0 + nw])
                                         for n0, nw in NSPLITS)
                        for n0, nw, pseg in segs:
                            for ki in range(KI):
                                base = (ki * NCH + mo * (NCH // MO)) * 128
                                lhs = stag[:, base + b:base + 1024:BPC]
                                nc.tensor.matmul(
                                    pseg[:] if psm_split else pseg,
                                    lhs, x_sb[:, ki, n0:n0 + nw],
                                    start=(ki == 0), stop=(ki == KI - 1),
                                )
                        if psm_split:
                            eng = (nc.vector.tensor_copy
                                   if evac_split and mo >= 1 else nc.scalar.copy)
                            eng(o_sb[:, mo, 0:512], ps_a[:])
                            eng(o_sb[:, mo, 512:HW], ps_b[:])
                        elif evac_split and mo >= 1:
                            nc.vector.tensor_copy(o_sb[:, mo, :], ps[:])
                        else:
                            nc.scalar.copy(o_sb[:, mo, :], ps[:])
                        if out_split_mo or b == BPC - 1:
                            cur_out_dmas.append(nc.sync.dma_start(
                                out_d[(0 if small_out else rep) * BPC + b,
                                      mo], o_sb[:, mo, :]))
                    if mo == MO - 1 and not (out_split_mo or b == BPC - 1):
                        cur_out_dmas.append(nc.sync.dma_start(
                            out_d[(0 if small_out else rep) * BPC + b]
                            .transpose([1, 0, 2]), o_sb[:]))
    nc.compile()
    return nc


def _host_prep(x, routing_weights, weight):
    """Full inputs -> per-core in_maps with the kernel's dram layouts."""
    # wt[ki][e*16+o16, chunk*128 + i_lo] = weight[e, chunk*16+o16, ki*128+i_lo]
    wt = np.ascontiguousarray(
        weight.reshape(E, NCH, OC, KI, 128)      # e, chunk, o16, ki, i_lo
        .transpose(3, 0, 2, 1, 4)                # ki, e, o16, chunk, i_lo
        .reshape(KI, 128, CPK).astype(np.float16))
    x_r = np.ascontiguousarray(x.reshape(B, KI, 128, HW).astype(np.float16))

    in_maps = []
    for c in range(N_CORES):
        r_core = routing_weights[c * BPC:(c + 1) * BPC]   # [BPC, E]
        rq = np.zeros((E, OC, OC, BPC), dtype=np.float16)
        for o16 in range(OC):
            rq[:, o16, o16, :] = r_core.T.astype(np.float16)
        in_maps.append({
            "x": x_r[c * BPC:(c + 1) * BPC],
            "rq": np.ascontiguousarray(rq.reshape(128, 128)),
            "wt": wt,
        })
    return in_maps


def kernel(x: np.ndarray, routing_weights: np.ndarray, weight: np.ndarray,
           _trace: bool = False):
    from concourse.bass_utils import run_bass_kernel_spmd

    x = np.asarray(x, dtype=np.float32)
    routing_weights = np.ascontiguousarray(
        np.asarray(routing_weights, dtype=np.float32))
    weight = np.asarray(weight, dtype=np.float32)

    if "nc" not in _cache:
        _cache["nc"] = _build()
    nc = _cache["nc"]

    in_maps = _host_prep(x, routing_weights, weight)
    res = run_bass_kernel_spmd(nc, in_maps, core_ids=list(range(N_CORES)),
                               trace=_trace)
    out = np.concatenate([res.results[c]["out"] for c in range(N_CORES)],
                         axis=0)
    if _trace:
        _cache["last_result"] = res
    return out.reshape(B, C_OUT, H, W).astype(np.float32)


if __name__ == "__main__":
    rng = np.random.default_rng(0)
    x = rng.standard_normal((B, C_IN, H, W), dtype=np.float32)
    rw = rng.random((B, E), dtype=np.float32)
    w = rng.standard_normal((E, C_OUT, C_IN), dtype=np.float32)
    got = kernel(x, rw, w)
    agg = np.einsum('be,eoi->boi', rw, w)
    want = np.einsum('boi,bihw->bohw', agg, x.reshape(B, C_IN, H, W))
    err = np.abs(got - want).max() / np.abs(want).max()
    print("rel err:", err)
